# revision 1
# baseline (speedup 1.0000x reference)
"""Trainium2 Bass kernel for the constrained leaky RNN.

Model (reference):
    x_proj = inputs @ W_in.T + b_in                    # [B, T, H]
    h_t    = (1-a)*h_{t-1} + a*tanh(x_proj_t + h_{t-1} @ W_rec.T + h_bias)
    out    = hs @ W_out.T + b_out                      # [B, T, O]
with B=64, T=2048, I=128, H=512, O=64, a=0.2.

Strategy:
  - Data-parallel over batch: 8 cores x 8 batch rows, no collectives.
  - All on-chip state kept transposed: H on partitions (4 tiles of 128),
    batch (8) on the free dim -> per-step elementwise ops are [128, 4*8].
  - State rescale trick: store Hs = h / a. Then
        Hs_t = 0.8 * Hs_{t-1} + tanh(...)      (one fused scalar_tensor_tensor)
    with a folded into W_rec and W_out on the host.
  - Per step: 16 matmuls (K=128 tiles of 0.2*W_rec^T as stationary, 8-col
    rhs = previous state), one DVE add (psum + x_proj), one ACT tanh, one
    fused DVE blend.
  - x_proj precomputed per 256-step chunk (batched matmuls, bias folded in
    via the ACT copy), output projection batched per chunk.
"""

import os
import sys

sys.path.insert(0, "/opt/trn_rl_repo")

import numpy as np

B, T, I, H, O = 64, 2048, 128, 512, 64
NCORES = 8
BL = B // NCORES          # batch rows per core
ALPHA = 0.2
DECAY = 1.0 - ALPHA
TC = 256                  # steps per chunk
NCHUNK = T // TC
SUB = 64                  # steps per psum group in projections (64*8 = 512 cols)

DT_REC = "fp16"           # "fp32" | "bf16" | "fp16": W_rec / state / W_out path
REPEAT = 1                # timing amplification: run the whole computation N times

_BUILD_CACHE = {}


def _build(dt_flag: str):
    import concourse.bass as bass
    import concourse.tile as tile
    from concourse import bacc, mybir
    from contextlib import ExitStack

    f32 = mybir.dt.float32
    dt_rec = {"fp32": f32, "bf16": mybir.dt.bfloat16,
              "fp16": mybir.dt.float16}[dt_flag]
    Alu = mybir.AluOpType
    Act = mybir.ActivationFunctionType

    nc = bacc.Bacc("TRN2")
    xT = nc.dram_tensor("xT", [I, T * BL], f32, kind="ExternalInput")
    wrecT = nc.dram_tensor("wrecT", [H, H], dt_rec, kind="ExternalInput")
    winT = nc.dram_tensor("winT", [I, H], f32, kind="ExternalInput")
    bias_c = nc.dram_tensor("bias_c", [128, 4], f32, kind="ExternalInput")
    woutT = nc.dram_tensor("woutT", [H, O], dt_rec, kind="ExternalInput")
    bout = nc.dram_tensor("bout", [O, 1], f32, kind="ExternalInput")
    outT = nc.dram_tensor("outT", [O, T * BL], f32, kind="ExternalOutput")

    with ExitStack() as ctx:
        tc = ctx.enter_context(tile.TileContext(nc))
        const = ctx.enter_context(tc.tile_pool(name="const", bufs=1))
        xpool = ctx.enter_context(tc.tile_pool(name="xpool", bufs=2))
        xppool = ctx.enter_context(tc.tile_pool(name="xppool", bufs=2))
        hpool = ctx.enter_context(tc.tile_pool(name="hpool", bufs=2))
        tmp = ctx.enter_context(tc.tile_pool(name="tmp", bufs=4))
        opool = ctx.enter_context(tc.tile_pool(name="opool", bufs=2))
        ps_h = ctx.enter_context(tc.tile_pool(name="ps_h", bufs=2, space="PSUM"))
        ps_xp = ctx.enter_context(tc.tile_pool(name="ps_xp", bufs=2, space="PSUM"))
        ps_o = ctx.enter_context(tc.tile_pool(name="ps_o", bufs=2, space="PSUM"))

        # ---- constants ----
        wrec_sb = const.tile([128, 4, H], dt_rec)       # [:, i, j*128+m]
        for i in range(4):
            nc.sync.dma_start(wrec_sb[:, i], wrecT[i * 128:(i + 1) * 128, :])
        win_sb = const.tile([I, H], f32)
        nc.sync.dma_start(win_sb, winT[:, :])
        bias_sb = const.tile([128, 4], f32)
        nc.sync.dma_start(bias_sb, bias_c[:, :])
        wout_sb = const.tile([128, 4, O], dt_rec)
        for j in range(4):
            nc.sync.dma_start(wout_sb[:, j], woutT[j * 128:(j + 1) * 128, :])
        bout_sb = const.tile([O, 1], f32)
        nc.sync.dma_start(bout_sb, bout[:, :])

        h_init = const.tile([128, 4, BL], dt_rec)
        nc.any.memzero(h_init[:])

        for _rep in range(REPEAT):
          prev = h_init[:]                               # state at t-1: [128, 4, BL]
          for c in range(NCHUNK):
            x_sb = xpool.tile([I, TC * BL], f32, tag="x")
            nc.sync.dma_start(x_sb, xT[:, c * TC * BL:(c + 1) * TC * BL])

            # x_proj for the chunk: xp[p, t, j, b] (+ combined bias)
            xp_sb = xppool.tile([128, TC, 4, BL], f32, tag="xp")
            for sub in range(TC // SUB):
                for j in range(4):
                    psx = ps_xp.tile([128, SUB * BL], f32, tag="psxp")
                    nc.tensor.matmul(
                        psx,
                        win_sb[:, j * 128:(j + 1) * 128],
                        x_sb[:, sub * SUB * BL:(sub + 1) * SUB * BL],
                        start=True, stop=True,
                    )
                    nc.scalar.activation(
                        xp_sb[:, sub * SUB:(sub + 1) * SUB, j, :],
                        psx.rearrange("p (t b) -> p t b", b=BL),
                        Act.Identity,
                        bias=bias_sb[:, j:j + 1],
                    )

            # recurrence
            hs = hpool.tile([128, TC, 4, BL], dt_rec, tag="hs")
            for t in range(TC):
                ps = ps_h.tile([128, 4 * BL], f32, tag="psh")
                psv = ps.rearrange("p (j b) -> p j b", b=BL)
                for j in range(4):
                    for i in range(4):
                        nc.tensor.matmul(
                            psv[:, j],
                            wrec_sb[:, i, j * 128:(j + 1) * 128],
                            prev[:, i],
                            start=(i == 0), stop=(i == 3),
                        )
                pre = tmp.tile([128, 4, BL], f32, tag="pre")
                nc.vector.tensor_add(out=pre[:], in0=psv, in1=xp_sb[:, t])
                g = tmp.tile([128, 4, BL], dt_rec, tag="g")
                nc.scalar.activation(g[:], pre[:], Act.Tanh)
                nc.vector.scalar_tensor_tensor(
                    out=hs[:, t], in0=prev, scalar=DECAY, in1=g[:],
                    op0=Alu.mult, op1=Alu.add,
                )
                prev = hs[:, t]

            # output projection for the chunk
            for sub in range(TC // SUB):
                pso = ps_o.tile([O, SUB * BL], f32, tag="pso")
                for j in range(4):
                    nc.tensor.matmul(
                        pso,
                        wout_sb[:, j],
                        hs[:, sub * SUB:(sub + 1) * SUB, j, :],
                        start=(j == 0), stop=(j == 3),
                    )
                ob = opool.tile([O, SUB * BL], f32, tag="ob")
                nc.scalar.activation(ob[:], pso[:], Act.Identity,
                                     bias=bout_sb[:, 0:1])
                nc.sync.dma_start(
                    outT[:, (c * TC + sub * SUB) * BL:(c * TC + (sub + 1) * SUB) * BL],
                    ob[:],
                )

    nc.finalize()
    return nc


def _get_nc(dt_flag: str):
    if dt_flag not in _BUILD_CACHE:
        _BUILD_CACHE[dt_flag] = _build(dt_flag)
    return _BUILD_CACHE[dt_flag]


def _prep_in_maps(inputs, dt_flag: str):
    import ml_dtypes

    x = np.asarray(inputs["inputs"], dtype=np.float32)
    W_in = np.asarray(inputs["W_in"], dtype=np.float32)
    b_in = np.asarray(inputs["b_in"], dtype=np.float32)
    W_rec = np.asarray(inputs["W_rec"], dtype=np.float32)
    h_bias = np.asarray(inputs["h_bias"], dtype=np.float32)
    W_out = np.asarray(inputs["W_out"], dtype=np.float32)
    b_out = np.asarray(inputs["b_out"], dtype=np.float32)

    dt = {"fp32": np.float32, "bf16": ml_dtypes.bfloat16,
          "fp16": np.float16}[dt_flag]
    wrecT = np.ascontiguousarray((ALPHA * W_rec.T).astype(dt))
    winT = np.ascontiguousarray(W_in.T)
    bias_c = np.ascontiguousarray((b_in + h_bias).reshape(4, 128).T)
    woutT = np.ascontiguousarray((ALPHA * W_out.T).astype(dt))
    bout = np.ascontiguousarray(b_out.reshape(O, 1))

    in_maps = []
    for c in range(NCORES):
        xc = x[c * BL:(c + 1) * BL]                     # [BL, T, I]
        xTc = np.ascontiguousarray(xc.transpose(2, 1, 0).reshape(I, T * BL))
        in_maps.append({
            "xT": xTc, "wrecT": wrecT, "winT": winT,
            "bias_c": bias_c, "woutT": woutT, "bout": bout,
        })
    return in_maps


def _run(inputs, trace=False, dt_flag=None, tmpdir=None):
    from concourse import bass_utils

    if dt_flag is None:
        dt_flag = DT_REC
    nc = _get_nc(dt_flag)
    in_maps = _prep_in_maps(inputs, dt_flag)
    res = bass_utils.run_bass_kernel_spmd(
        nc, in_maps, core_ids=list(range(NCORES)), trace=trace, tmpdir=tmpdir,
    )
    outs = []
    for c in range(NCORES):
        oT = res.results[c]["outT"]                     # [O, T*BL]
        outs.append(oT.reshape(O, T, BL).transpose(2, 1, 0))
    full = np.concatenate(outs, axis=0).astype(np.float32)
    return full, res


def kernel(**inputs) -> np.ndarray:
    out, _ = _run(inputs, trace=False)
    return out



# revision 5
# speedup vs baseline: 1.0003x; 1.0003x over previous
"""Trainium2 Bass kernel for the constrained leaky RNN.

Model (reference):
    x_proj = inputs @ W_in.T + b_in                    # [B, T, H]
    h_t    = (1-a)*h_{t-1} + a*tanh(x_proj_t + h_{t-1} @ W_rec.T + h_bias)
    out    = hs @ W_out.T + b_out                      # [B, T, O]
with B=64, T=2048, I=128, H=512, O=64, a=0.2.

Strategy:
  - Data-parallel over batch: 8 cores x 8 batch rows, no collectives.
  - All on-chip state kept transposed: H on partitions (4 tiles of 128),
    batch (8) on the free dim -> per-step elementwise ops are [128, 4*8].
  - State rescale trick: store Hs = h / a. Then
        Hs_t = 0.8 * Hs_{t-1} + tanh(...)      (one fused scalar_tensor_tensor)
    with a folded into W_rec and W_out on the host.
  - Per step: 16 matmuls (K=128 tiles of 0.2*W_rec^T as stationary, 8-col
    rhs = previous state), one DVE add (psum + x_proj), one ACT tanh, one
    fused DVE blend.
  - x_proj precomputed per 256-step chunk (batched matmuls, bias folded in
    via the ACT copy), output projection batched per chunk.
"""

import os
import sys

sys.path.insert(0, "/opt/trn_rl_repo")

import numpy as np

B, T, I, H, O = 64, 2048, 128, 512, 64
NCORES = 8
BL = B // NCORES          # batch rows per core
ALPHA = 0.2
DECAY = 1.0 - ALPHA
TC = 256                  # steps per chunk
NCHUNK = T // TC
SUB = 64                  # steps per psum group in projections (64*8 = 512 cols)

DT_REC = "bf16"           # "fp32" | "bf16" | "fp16": W_rec / state / W_out path
REPEAT = 1                # timing amplification: run the whole computation N times

_BUILD_CACHE = {}
_LDWOPT_PATCHED = False


def _enable_ldw_opt():
    """walrus codegen defaults --enable-ldw-opt=true but concourse's caller
    pins it false; the recurrence here is LDWEIGHTS-streaming-bound, so flip
    it back on for this kernel's compile."""
    global _LDWOPT_PATCHED
    if _LDWOPT_PATCHED:
        return
    from concourse import bass_utils as _bu

    _orig = _bu.run_command

    def _patched(argv, **kwargs):
        argv = [
            "--enable-ldw-opt=true" if a == "--enable-ldw-opt=false" else a
            for a in argv
        ]
        return _orig(argv, **kwargs)

    _bu.run_command = _patched
    _LDWOPT_PATCHED = True


def _build(dt_flag: str):
    import concourse.bass as bass
    import concourse.tile as tile
    from concourse import bacc, mybir
    from contextlib import ExitStack

    f32 = mybir.dt.float32
    dt_rec = {"fp32": f32, "bf16": mybir.dt.bfloat16,
              "fp16": mybir.dt.float16}[dt_flag]
    Alu = mybir.AluOpType
    Act = mybir.ActivationFunctionType

    nc = bacc.Bacc("TRN2")
    xT = nc.dram_tensor("xT", [I, T * BL], f32, kind="ExternalInput")
    wrecT = nc.dram_tensor("wrecT", [H, H], dt_rec, kind="ExternalInput")
    winT = nc.dram_tensor("winT", [I, H], f32, kind="ExternalInput")
    bias_c = nc.dram_tensor("bias_c", [128, 4], f32, kind="ExternalInput")
    woutT = nc.dram_tensor("woutT", [H, O], dt_rec, kind="ExternalInput")
    bout = nc.dram_tensor("bout", [O, 1], f32, kind="ExternalInput")
    outT = nc.dram_tensor("outT", [O, T * BL], f32, kind="ExternalOutput")

    with ExitStack() as ctx:
        tc = ctx.enter_context(tile.TileContext(nc))
        const = ctx.enter_context(tc.tile_pool(name="const", bufs=1))
        xpool = ctx.enter_context(tc.tile_pool(name="xpool", bufs=2))
        xppool = ctx.enter_context(tc.tile_pool(name="xppool", bufs=2))
        hpool = ctx.enter_context(tc.tile_pool(name="hpool", bufs=2))
        tmp = ctx.enter_context(tc.tile_pool(name="tmp", bufs=4))
        opool = ctx.enter_context(tc.tile_pool(name="opool", bufs=2))
        ps_h = ctx.enter_context(tc.tile_pool(name="ps_h", bufs=2, space="PSUM"))
        ps_xp = ctx.enter_context(tc.tile_pool(name="ps_xp", bufs=2, space="PSUM"))
        ps_o = ctx.enter_context(tc.tile_pool(name="ps_o", bufs=2, space="PSUM"))

        # ---- constants ----
        wrec_sb = const.tile([128, 4, H], dt_rec)       # [:, i, j*128+m]
        for i in range(4):
            nc.sync.dma_start(wrec_sb[:, i], wrecT[i * 128:(i + 1) * 128, :])
        win_sb = const.tile([I, H], f32)
        nc.sync.dma_start(win_sb, winT[:, :])
        bias_sb = const.tile([128, 4], f32)
        nc.sync.dma_start(bias_sb, bias_c[:, :])
        wout_sb = const.tile([128, 4, O], dt_rec)
        for j in range(4):
            nc.sync.dma_start(wout_sb[:, j], woutT[j * 128:(j + 1) * 128, :])
        bout_sb = const.tile([O, 1], f32)
        nc.sync.dma_start(bout_sb, bout[:, :])

        h_init = const.tile([128, 4, BL], dt_rec)
        nc.any.memzero(h_init[:])

        for _rep in range(REPEAT):
          prev = h_init[:]                               # state at t-1: [128, 4, BL]
          for c in range(NCHUNK):
            x_sb = xpool.tile([I, TC * BL], f32, tag="x")
            nc.sync.dma_start(x_sb, xT[:, c * TC * BL:(c + 1) * TC * BL])

            # x_proj for the chunk: xp[p, t, j, b] (+ combined bias)
            xp_sb = xppool.tile([128, TC, 4, BL], f32, tag="xp")
            for sub in range(TC // SUB):
                for j in range(4):
                    psx = ps_xp.tile([128, SUB * BL], f32, tag="psxp")
                    nc.tensor.matmul(
                        psx,
                        win_sb[:, j * 128:(j + 1) * 128],
                        x_sb[:, sub * SUB * BL:(sub + 1) * SUB * BL],
                        start=True, stop=True,
                    )
                    nc.scalar.activation(
                        xp_sb[:, sub * SUB:(sub + 1) * SUB, j, :],
                        psx.rearrange("p (t b) -> p t b", b=BL),
                        Act.Identity,
                        bias=bias_sb[:, j:j + 1],
                    )

            # recurrence
            hs = hpool.tile([128, TC, 4, BL], dt_rec, tag="hs")
            for t in range(TC):
                ps = ps_h.tile([128, 4 * BL], f32, tag="psh")
                psv = ps.rearrange("p (j b) -> p j b", b=BL)
                for j in range(4):
                    for i in range(4):
                        nc.tensor.matmul(
                            psv[:, j],
                            wrec_sb[:, i, j * 128:(j + 1) * 128],
                            prev[:, i],
                            start=(i == 0), stop=(i == 3),
                        )
                pre = tmp.tile([128, 4, BL], f32, tag="pre")
                nc.vector.tensor_add(out=pre[:], in0=psv, in1=xp_sb[:, t])
                g = tmp.tile([128, 4, BL], dt_rec, tag="g")
                nc.scalar.activation(g[:], pre[:], Act.Tanh)
                nc.vector.scalar_tensor_tensor(
                    out=hs[:, t], in0=prev, scalar=DECAY, in1=g[:],
                    op0=Alu.mult, op1=Alu.add,
                )
                prev = hs[:, t]

            # output projection for the chunk
            for sub in range(TC // SUB):
                pso = ps_o.tile([O, SUB * BL], f32, tag="pso")
                for j in range(4):
                    nc.tensor.matmul(
                        pso,
                        wout_sb[:, j],
                        hs[:, sub * SUB:(sub + 1) * SUB, j, :],
                        start=(j == 0), stop=(j == 3),
                    )
                ob = opool.tile([O, SUB * BL], f32, tag="ob")
                nc.scalar.activation(ob[:], pso[:], Act.Identity,
                                     bias=bout_sb[:, 0:1])
                nc.sync.dma_start(
                    outT[:, (c * TC + sub * SUB) * BL:(c * TC + (sub + 1) * SUB) * BL],
                    ob[:],
                )

    nc.finalize()
    return nc


def _get_nc(dt_flag: str):
    if dt_flag not in _BUILD_CACHE:
        _BUILD_CACHE[dt_flag] = _build(dt_flag)
    return _BUILD_CACHE[dt_flag]


def _prep_in_maps(inputs, dt_flag: str):
    import ml_dtypes

    x = np.asarray(inputs["inputs"], dtype=np.float32)
    W_in = np.asarray(inputs["W_in"], dtype=np.float32)
    b_in = np.asarray(inputs["b_in"], dtype=np.float32)
    W_rec = np.asarray(inputs["W_rec"], dtype=np.float32)
    h_bias = np.asarray(inputs["h_bias"], dtype=np.float32)
    W_out = np.asarray(inputs["W_out"], dtype=np.float32)
    b_out = np.asarray(inputs["b_out"], dtype=np.float32)

    dt = {"fp32": np.float32, "bf16": ml_dtypes.bfloat16,
          "fp16": np.float16}[dt_flag]
    wrecT = np.ascontiguousarray((ALPHA * W_rec.T).astype(dt))
    winT = np.ascontiguousarray(W_in.T)
    bias_c = np.ascontiguousarray((b_in + h_bias).reshape(4, 128).T)
    woutT = np.ascontiguousarray((ALPHA * W_out.T).astype(dt))
    bout = np.ascontiguousarray(b_out.reshape(O, 1))

    in_maps = []
    for c in range(NCORES):
        xc = x[c * BL:(c + 1) * BL]                     # [BL, T, I]
        xTc = np.ascontiguousarray(xc.transpose(2, 1, 0).reshape(I, T * BL))
        in_maps.append({
            "xT": xTc, "wrecT": wrecT, "winT": winT,
            "bias_c": bias_c, "woutT": woutT, "bout": bout,
        })
    return in_maps


def _run(inputs, trace=False, dt_flag=None, tmpdir=None):
    from concourse import bass_utils

    if dt_flag is None:
        dt_flag = DT_REC
    nc = _get_nc(dt_flag)
    in_maps = _prep_in_maps(inputs, dt_flag)
    res = bass_utils.run_bass_kernel_spmd(
        nc, in_maps, core_ids=list(range(NCORES)), trace=trace, tmpdir=tmpdir,
    )
    outs = []
    for c in range(NCORES):
        oT = res.results[c]["outT"]                     # [O, T*BL]
        outs.append(oT.reshape(O, T, BL).transpose(2, 1, 0))
    full = np.concatenate(outs, axis=0).astype(np.float32)
    return full, res


def kernel(**inputs) -> np.ndarray:
    out, _ = _run(inputs, trace=False)
    return out



# revision 10
# speedup vs baseline: 1.2374x; 1.2370x over previous
"""Trainium2 Bass kernel for the constrained leaky RNN.

Model (reference):
    x_proj = inputs @ W_in.T + b_in                    # [B, T, H]
    h_t    = (1-a)*h_{t-1} + a*tanh(x_proj_t + h_{t-1} @ W_rec.T + h_bias)
    out    = hs @ W_out.T + b_out                      # [B, T, O]
with B=64, T=2048, I=128, H=512, O=64, a=0.2.

Strategy (v2):
  - Data-parallel over batch: 8 cores x 8 batch rows, no collectives.
  - State transposed on-chip: H on partitions (4 tiles of 128), batch (8)
    on the free dim.
  - Critical-path restructure: with q_t = h_t @ W_rec.T,
        pre_{t+1} = (xp_{t+1} - 0.8 xp_t) + 0.8 pre_t + g_t @ (0.2 W_rec).T
        g_t = tanh(pre_t)
    so the recurrent matmuls consume the tanh output g_t directly and the
    leaky blend (Hs_t = 0.8 Hs_{t-1} + g_t, needed only for the output
    projection) moves off the serial chain.
  - Per step: 4 x-diff matmuls start a fresh full-bank PSUM tile, 16 W_rec
    matmuls accumulate on top; one fused STT (split in 2 halves) forms
    pre_{t+1} in PSUM; tanh reads PSUM (ScalarE's fast port), split into
    two halves that unblock the recurrent matmuls incrementally.
  - x differencing (xd_t = x_t - 0.8 x_{t-1}) done on the host; bias
    enters via a tiny K=1 rank-1 matmul only when nonzero.
  - Output projection batched per 32-step chunk from the Hs buffer.
"""

import os
import sys

sys.path.insert(0, "/opt/trn_rl_repo")

import numpy as np

B, T, I, H, O = 64, 2048, 128, 512, 64
NCORES = 8
BL = B // NCORES          # batch rows per core
ALPHA = 0.2
DECAY = 1.0 - ALPHA
TC = 32                   # steps per chunk (xc DMA / hs buffer / outproj)
NCHUNK = T // TC

DT_REC = "fp16"           # weights/state dtype on chip
SCALE = 1.0               # scale folded into W_in/W_rec/beta; tanh scale=1/S

_BUILD_CACHE = {}


def _build(dt_flag: str, with_beta: bool):
    import concourse.tile as tile
    from concourse import bacc, mybir
    from contextlib import ExitStack

    f32 = mybir.dt.float32
    dt_rec = {"fp32": f32, "bf16": mybir.dt.bfloat16,
              "fp16": mybir.dt.float16}[dt_flag]
    Alu = mybir.AluOpType
    Act = mybir.ActivationFunctionType

    nc = bacc.Bacc("TRN2")
    xT = nc.dram_tensor("xT", [I, T * BL], dt_rec, kind="ExternalInput")
    wrecT = nc.dram_tensor("wrecT", [H, H], dt_rec, kind="ExternalInput")
    winT = nc.dram_tensor("winT", [I, H], dt_rec, kind="ExternalInput")
    beta = nc.dram_tensor("beta", [1, H], dt_rec, kind="ExternalInput")
    woutT = nc.dram_tensor("woutT", [H, O], dt_rec, kind="ExternalInput")
    bout = nc.dram_tensor("bout", [O, 1], f32, kind="ExternalInput")
    outT = nc.dram_tensor("outT", [O, T * BL], f32, kind="ExternalOutput")

    inv_s = 1.0 / SCALE

    with ExitStack() as ctx:
        tc = ctx.enter_context(tile.TileContext(nc))
        const = ctx.enter_context(tc.tile_pool(name="const", bufs=1))
        xpool = ctx.enter_context(tc.tile_pool(name="xpool", bufs=2))
        gpool = ctx.enter_context(tc.tile_pool(name="gpool", bufs=4))
        hspool = ctx.enter_context(tc.tile_pool(name="hspool", bufs=2))
        opool = ctx.enter_context(tc.tile_pool(name="opool", bufs=2))
        prepool = ctx.enter_context(tc.tile_pool(name="prepool", bufs=4))
        # full-bank PSUM tiles: 4 (recurrence) + 2 (outproj)
        ps_rec = ctx.enter_context(tc.tile_pool(name="ps_rec", bufs=4, space="PSUM"))
        ps_o = ctx.enter_context(tc.tile_pool(name="ps_o", bufs=2, space="PSUM"))

        # ---- constants ----
        wrec_sb = const.tile([128, 4, H], dt_rec)       # [:, i, j*128+m]
        for i in range(4):
            nc.sync.dma_start(wrec_sb[:, i], wrecT[i * 128:(i + 1) * 128, :])
        win_sb = const.tile([I, H], dt_rec)
        nc.sync.dma_start(win_sb, winT[:, :])
        wout_sb = const.tile([128, 4, O], dt_rec)
        for j in range(4):
            nc.sync.dma_start(wout_sb[:, j], woutT[j * 128:(j + 1) * 128, :])
        bout_sb = const.tile([O, 1], f32)
        nc.sync.dma_start(bout_sb, bout[:, :])
        if with_beta:
            beta_sb = const.tile([1, H], dt_rec)
            nc.sync.dma_start(beta_sb, beta[:, :])
            bcoef_sb = const.tile([1, 2, BL], dt_rec)
            nc.any.memset(bcoef_sb[:, 0], 1.0)
            nc.any.memset(bcoef_sb[:, 1], ALPHA)

        hs_init = const.tile([128, 4, BL], dt_rec)
        nc.any.memzero(hs_init[:])

        x_tiles = {}

        def load_chunk(c):
            if c >= NCHUNK:
                return
            xt = xpool.tile([I, TC * BL], dt_rec, tag="x")
            nc.sync.dma_start(xt, xT[:, c * TC * BL:(c + 1) * TC * BL])
            x_tiles[c] = xt

        load_chunk(0)
        load_chunk(1)

        def xd_matmuls(t):
            """Fresh (full-bank) psum tile for step t with x-diff (+beta)."""
            bank = ps_rec.tile([128, 16, 4, BL], f32, tag="psrec")
            ps = bank[:, 0]                             # [128, 4, BL]
            c, tl = divmod(t, TC)
            xc = x_tiles[c]
            for j in range(4):
                nc.tensor.matmul(
                    ps[:, j],
                    win_sb[:, j * 128:(j + 1) * 128],
                    xc[:, tl * BL:(tl + 1) * BL],
                    start=(j == 0), stop=False,
                    skip_group_check=True,
                )
            if with_beta:
                sel = 0 if t == 0 else 1
                for j in range(4):
                    nc.tensor.matmul(
                        ps[:, j],
                        beta_sb[:, j * 128:(j + 1) * 128],
                        bcoef_sb[:, sel],
                        start=False, stop=False,
                        skip_group_check=True,
                    )
            return ps

        ps0 = xd_matmuls(0)       # step 0: xp only (h_{-1} = 0)
        # copy to SBUF so step 0 is uniform (pre state lives in SBUF)
        pre_t = prepool.tile([128, 4, BL], f32, tag="pre")
        nc.scalar.activation(pre_t[:], ps0, Act.Identity)
        hs_prev = hs_init[:]
        hs_chunk = None

        for t in range(T):
            c, tl = divmod(t, TC)
            if tl == 0:
                hs_chunk = hspool.tile([128, 4, TC, BL], dt_rec, tag="hs")
                if t > 0:
                    load_chunk(c + 1)

            # g_t = tanh(pre_t / S), two halves so matmuls start early
            g = gpool.tile([128, 4, BL], dt_rec, tag="g")
            src = pre_t
            nc.scalar.activation(g[:, 0:2], src[:, 0:2], Act.Tanh, scale=inv_s)
            nc.scalar.activation(g[:, 2:4], src[:, 2:4], Act.Tanh, scale=inv_s)

            if t < T - 1:
                ps_next = xd_matmuls(t + 1)
                # recurrent matmuls: += g_t @ (a W_rec).T, phase by tanh half
                for phase in range(2):
                    for j in range(4):
                        for i in (2 * phase, 2 * phase + 1):
                            nc.tensor.matmul(
                                ps_next[:, j],
                                wrec_sb[:, i, j * 128:(j + 1) * 128],
                                g[:, i],
                                start=False,
                                stop=(phase == 1 and i == 3),
                                skip_group_check=True,
                            )

                # pre_{t+1} = 0.8 * pre_t + ps_next   (two halves, SBUF out)
                pre_next = prepool.tile([128, 4, BL], f32, tag="pre")
                for hf in range(2):
                    sl = slice(2 * hf, 2 * hf + 2)
                    nc.vector.scalar_tensor_tensor(
                        out=pre_next[:, sl],
                        in0=src[:, sl], scalar=DECAY,
                        in1=ps_next[:, sl],
                        op0=Alu.mult, op1=Alu.add,
                    )
                pre_t = pre_next

            # Hs_t = 0.8 * Hs_{t-1} + g_t   (off critical path)
            nc.vector.scalar_tensor_tensor(
                out=hs_chunk[:, :, tl], in0=hs_prev, scalar=DECAY,
                in1=g[:], op0=Alu.mult, op1=Alu.add,
            )
            hs_prev = hs_chunk[:, :, tl]

            if tl == TC - 1:
                pso_bank = ps_o.tile([O, 2, TC * BL], f32, tag="pso")
                pso = pso_bank[:, 0]
                for j in range(4):
                    nc.tensor.matmul(
                        pso,
                        wout_sb[:, j],
                        hs_chunk[:, j],
                        start=(j == 0), stop=(j == 3),
                    )
                ob = opool.tile([O, TC * BL], f32, tag="ob")
                nc.scalar.activation(ob[:], pso, Act.Identity,
                                     bias=bout_sb[:, 0:1])
                nc.sync.dma_start(
                    outT[:, c * TC * BL:(c + 1) * TC * BL], ob[:],
                )

    nc.finalize()
    return nc


def _get_nc(dt_flag: str, with_beta: bool):
    key = (dt_flag, with_beta)
    if key not in _BUILD_CACHE:
        _BUILD_CACHE[key] = _build(dt_flag, with_beta)
    return _BUILD_CACHE[key]


def _prep_in_maps(inputs, dt_flag: str):
    import ml_dtypes

    x = np.asarray(inputs["inputs"], dtype=np.float32)
    W_in = np.asarray(inputs["W_in"], dtype=np.float32)
    b_in = np.asarray(inputs["b_in"], dtype=np.float32)
    W_rec = np.asarray(inputs["W_rec"], dtype=np.float32)
    h_bias = np.asarray(inputs["h_bias"], dtype=np.float32)
    W_out = np.asarray(inputs["W_out"], dtype=np.float32)
    b_out = np.asarray(inputs["b_out"], dtype=np.float32)

    dt = {"fp32": np.float32, "bf16": ml_dtypes.bfloat16,
          "fp16": np.float16}[dt_flag]

    # x differencing: xd_0 = x_0 ; xd_t = x_t - 0.8 x_{t-1}
    xd = x.copy()
    xd[:, 1:] -= DECAY * x[:, :-1]

    wrecT = np.ascontiguousarray((SCALE * ALPHA * W_rec.T).astype(dt))
    winT = np.ascontiguousarray((SCALE * W_in.T).astype(dt))
    beta_v = np.ascontiguousarray(
        (SCALE * (b_in + h_bias)).astype(dt).reshape(1, H))
    with_beta = bool(np.any(np.asarray(beta_v, dtype=np.float32) != 0))
    woutT = np.ascontiguousarray((ALPHA * W_out.T).astype(dt))
    bout = np.ascontiguousarray(b_out.reshape(O, 1))

    in_maps = []
    for c in range(NCORES):
        xc = xd[c * BL:(c + 1) * BL]                    # [BL, T, I]
        xTc = np.ascontiguousarray(
            xc.transpose(2, 1, 0).reshape(I, T * BL).astype(dt))
        in_maps.append({
            "xT": xTc, "wrecT": wrecT, "winT": winT,
            "beta": beta_v, "woutT": woutT, "bout": bout,
        })
    return in_maps, with_beta


def _run(inputs, trace=False, dt_flag=None, tmpdir=None):
    from concourse import bass_utils

    if dt_flag is None:
        dt_flag = DT_REC
    in_maps, with_beta = _prep_in_maps(inputs, dt_flag)
    nc = _get_nc(dt_flag, with_beta)
    res = bass_utils.run_bass_kernel_spmd(
        nc, in_maps, core_ids=list(range(NCORES)), trace=trace, tmpdir=tmpdir,
    )
    outs = []
    for c in range(NCORES):
        oT = res.results[c]["outT"]                     # [O, T*BL]
        outs.append(oT.reshape(O, T, BL).transpose(2, 1, 0))
    full = np.concatenate(outs, axis=0).astype(np.float32)
    return full, res


def kernel(**inputs) -> np.ndarray:
    out, _ = _run(inputs, trace=False)
    return out


# revision 14
# speedup vs baseline: 1.3484x; 1.0897x over previous
"""Trainium2 Bass kernel for the constrained leaky RNN.

Model (reference):
    x_proj = inputs @ W_in.T + b_in                    # [B, T, H]
    h_t    = (1-a)*h_{t-1} + a*tanh(x_proj_t + h_{t-1} @ W_rec.T + h_bias)
    out    = hs @ W_out.T + b_out                      # [B, T, O]
with B=64, T=2048, I=128, H=512, O=64, a=0.2.

Strategy (v3):
  - Data-parallel over batch: 8 cores x 8 batch rows, no collectives.
  - State transposed on-chip: H on partitions (4 tiles of 128), batch (8)
    on the free dim.
  - The whole per-step serial chain is reduced to: psum slot closes ->
    tanh (reads PSUM directly) -> recurrent matmuls. Everything else is
    off the critical path:
      slot_{t+1} = xd-proj_{t+1} (+beta) + 0.8*pre_t (identity matmul)
                   + g_t @ (a W_rec).T
      g_t   = tanh(slot_t / S)           [slot_t == pre_t by construction]
      pre_t = fp16 copy of slot_t to SBUF (DVE, off-chain; feeds the
              0.8*I matmul of step t+1)
    where xd_t = x_t - 0.8 x_{t-1} is differenced on the host so that the
    0.8-decay of the input projection telescopes into the identity matmul.
  - Each step's slot is split across TWO psum banks (A: j0,j1 / B: j2,j3)
    because Tile tracks PSUM dependencies at bank granularity: tanh half A
    fires as soon as bank A closes, overlapping the remaining matmuls.
  - h_t is reconstructed off-chain (Hs_t = 0.8 Hs_{t-1} + g_t on DVE) and
    the output projection is batched per 32-step chunk, its matmuls and
    PSUM-evacuation spread into the next chunk's early steps.
"""

import os
import sys

sys.path.insert(0, "/opt/trn_rl_repo")

import numpy as np

B, T, I, H, O = 64, 2048, 128, 512, 64
NCORES = 8
BL = B // NCORES          # batch rows per core
ALPHA = 0.2
DECAY = 1.0 - ALPHA
TC = 32                   # steps per chunk (xc DMA / hs buffer / outproj)
NCHUNK = T // TC

DT_REC = "fp16"           # weights/state dtype on chip
SCALE = 1.0               # scale folded into W_in/W_rec/beta; tanh scale=1/S

_BUILD_CACHE = {}


def _build(dt_flag: str, with_beta: bool):
    import concourse.tile as tile
    from concourse import bacc, mybir
    from contextlib import ExitStack

    f32 = mybir.dt.float32
    dt_rec = {"fp32": f32, "bf16": mybir.dt.bfloat16,
              "fp16": mybir.dt.float16}[dt_flag]
    Alu = mybir.AluOpType
    Act = mybir.ActivationFunctionType

    nc = bacc.Bacc("TRN2")
    xT = nc.dram_tensor("xT", [I, T * BL], dt_rec, kind="ExternalInput")
    wrecT = nc.dram_tensor("wrecT", [H, H], dt_rec, kind="ExternalInput")
    winT = nc.dram_tensor("winT", [I, H], dt_rec, kind="ExternalInput")
    id08 = nc.dram_tensor("id08", [128, 128], dt_rec, kind="ExternalInput")
    beta = nc.dram_tensor("beta", [1, H], dt_rec, kind="ExternalInput")
    woutT = nc.dram_tensor("woutT", [H, O], dt_rec, kind="ExternalInput")
    bout = nc.dram_tensor("bout", [O, 1], f32, kind="ExternalInput")
    outT = nc.dram_tensor("outT", [O, T * BL], f32, kind="ExternalOutput")

    inv_s = 1.0 / SCALE

    with ExitStack() as ctx:
        tc = ctx.enter_context(tile.TileContext(nc))
        const = ctx.enter_context(tc.tile_pool(name="const", bufs=1))
        xpool = ctx.enter_context(tc.tile_pool(name="xpool", bufs=2))
        gpool = ctx.enter_context(tc.tile_pool(name="gpool", bufs=4))
        prepool = ctx.enter_context(tc.tile_pool(name="prepool", bufs=4))
        hspool = ctx.enter_context(tc.tile_pool(name="hspool", bufs=2))
        opool = ctx.enter_context(tc.tile_pool(name="opool", bufs=2))
        # full-bank PSUM tiles: 2+2 (slot A/B double-buffered) + 2 (outproj)
        psA = ctx.enter_context(tc.tile_pool(name="psA", bufs=2, space="PSUM"))
        psB = ctx.enter_context(tc.tile_pool(name="psB", bufs=2, space="PSUM"))
        ps_o = ctx.enter_context(tc.tile_pool(name="ps_o", bufs=2, space="PSUM"))

        # ---- constants ----
        wrec_sb = const.tile([128, 4, H], dt_rec)       # [:, i, j*128+m]
        for i in range(4):
            nc.sync.dma_start(wrec_sb[:, i], wrecT[i * 128:(i + 1) * 128, :])
        win_sb = const.tile([I, H], dt_rec)
        nc.sync.dma_start(win_sb, winT[:, :])
        id_sb = const.tile([128, 128], dt_rec)
        nc.sync.dma_start(id_sb, id08[:, :])
        wout_sb = const.tile([128, 4, O], dt_rec)
        for j in range(4):
            nc.sync.dma_start(wout_sb[:, j], woutT[j * 128:(j + 1) * 128, :])
        bout_sb = const.tile([O, 1], f32)
        nc.sync.dma_start(bout_sb, bout[:, :])
        if with_beta:
            beta_sb = const.tile([1, H], dt_rec)
            nc.sync.dma_start(beta_sb, beta[:, :])
            bcoef_sb = const.tile([1, 2, BL], dt_rec)
            nc.any.memset(bcoef_sb[:, 0], 1.0)
            nc.any.memset(bcoef_sb[:, 1], ALPHA)

        hs_init = const.tile([128, 4, BL], dt_rec)
        nc.any.memzero(hs_init[:])

        x_tiles = {}

        def load_chunk(c):
            if c >= NCHUNK:
                return
            xt = xpool.tile([I, TC * BL], dt_rec, tag="x")
            nc.sync.dma_start(xt, xT[:, c * TC * BL:(c + 1) * TC * BL])
            x_tiles[c] = xt

        load_chunk(0)
        load_chunk(1)

        def slot_view(j, slots):
            """(slot_half, local_j) for output tile j."""
            return slots[j // 2], j % 2

        def xd_beta_matmuls(t, slots):
            """x-diff (+beta) matmuls opening step t's two slot halves."""
            c, tl = divmod(t, TC)
            xc = x_tiles[c]
            for j in range(4):
                sv, lj = slot_view(j, slots)
                nc.tensor.matmul(
                    sv[:, lj],
                    win_sb[:, j * 128:(j + 1) * 128],
                    xc[:, tl * BL:(tl + 1) * BL],
                    start=(lj == 0), stop=False,
                    skip_group_check=True,
                )
            if with_beta:
                sel = 0 if t == 0 else 1
                for j in range(4):
                    sv, lj = slot_view(j, slots)
                    nc.tensor.matmul(
                        sv[:, lj],
                        beta_sb[:, j * 128:(j + 1) * 128],
                        bcoef_sb[:, sel],
                        start=False, stop=False,
                        skip_group_check=True,
                    )

        def new_slots():
            a_full = psA.tile([128, 32, 2, BL], f32, tag="slotA")
            b_full = psB.tile([128, 32, 2, BL], f32, tag="slotB")
            return (a_full[:, 0], b_full[:, 0])

        def outproj(hsc, oc):
            pso = ps_o.tile([O, TC * BL], f32, tag="pso")
            for j in range(4):
                nc.tensor.matmul(pso, wout_sb[:, j], hsc[:, j],
                                 start=(j == 0), stop=(j == 3))
            ob = opool.tile([O, TC * BL], f32, tag="ob")
            nc.scalar.activation(ob[:], pso, Act.Identity,
                                 bias=bout_sb[:, 0:1])
            nc.sync.dma_start(outT[:, oc * TC * BL:(oc + 1) * TC * BL], ob[:])

        # step 0: xp only (h_{-1} = 0)
        slots_t = new_slots()
        xd_beta_matmuls(0, slots_t)
        hs_prev = hs_init[:]
        hs_chunk = None
        prev_hs_chunk = None

        for t in range(T):
            c, tl = divmod(t, TC)
            if tl == 0:
                prev_hs_chunk = hs_chunk
                hs_chunk = hspool.tile([128, 4, TC, BL], dt_rec, tag="hs")
                if t > 0:
                    load_chunk(c + 1)

            # g_t = tanh(slot_t / S): half A fires when bank A closes
            g = gpool.tile([128, 4, BL], dt_rec, tag="g")
            nc.scalar.activation(g[:, 0:2], slots_t[0], Act.Tanh, scale=inv_s)
            nc.scalar.activation(g[:, 2:4], slots_t[1], Act.Tanh, scale=inv_s)

            # off-chain: fp16 SBUF copy of slot_t (feeds step t+1's 0.8*I mm)
            pre_n = prepool.tile([128, 4, BL], dt_rec, tag="pre")
            nc.vector.tensor_scalar_mul(out=pre_n[:, 0:2], in0=slots_t[0],
                                        scalar1=1.0)
            nc.vector.tensor_scalar_mul(out=pre_n[:, 2:4], in0=slots_t[1],
                                        scalar1=1.0)

            # Hs_t = 0.8 * Hs_{t-1} + g_t   (off critical path)
            nc.vector.scalar_tensor_tensor(
                out=hs_chunk[:, :, tl], in0=hs_prev, scalar=DECAY,
                in1=g[:], op0=Alu.mult, op1=Alu.add,
            )
            hs_prev = hs_chunk[:, :, tl]

            if t < T - 1:
                slots_n = new_slots()
                xd_beta_matmuls(t + 1, slots_n)
                # recurrent matmuls += g_t @ (a W_rec).T
                # phase 1: i in {0,1} (after tanh half A); then the 0.8*pre_t
                # identity matmuls (whose DVE copy has completed by then);
                # phase 2: i in {2,3}, j ascending closes bank A (j0,j1) first.
                for phase in range(2):
                    if phase == 1:
                        for j in range(4):
                            sv, lj = slot_view(j, slots_n)
                            nc.tensor.matmul(
                                sv[:, lj], id_sb[:, :], pre_n[:, j],
                                start=False, stop=False,
                                skip_group_check=True,
                            )
                    for j in range(4):
                        sv, lj = slot_view(j, slots_n)
                        for i in (2 * phase, 2 * phase + 1):
                            nc.tensor.matmul(
                                sv[:, lj],
                                wrec_sb[:, i, j * 128:(j + 1) * 128],
                                g[:, i],
                                start=False,
                                stop=(phase == 1 and i == 3),
                                skip_group_check=True,
                            )
                slots_t = slots_n

            # deferred output projection for the previous chunk
            if tl == 2 and prev_hs_chunk is not None:
                outproj(prev_hs_chunk, c - 1)

        outproj(hs_chunk, NCHUNK - 1)

    nc.finalize()
    return nc


def _get_nc(dt_flag: str, with_beta: bool):
    key = (dt_flag, with_beta)
    if key not in _BUILD_CACHE:
        _BUILD_CACHE[key] = _build(dt_flag, with_beta)
    return _BUILD_CACHE[key]


def _prep_in_maps(inputs, dt_flag: str):
    import ml_dtypes

    x = np.asarray(inputs["inputs"], dtype=np.float32)
    W_in = np.asarray(inputs["W_in"], dtype=np.float32)
    b_in = np.asarray(inputs["b_in"], dtype=np.float32)
    W_rec = np.asarray(inputs["W_rec"], dtype=np.float32)
    h_bias = np.asarray(inputs["h_bias"], dtype=np.float32)
    W_out = np.asarray(inputs["W_out"], dtype=np.float32)
    b_out = np.asarray(inputs["b_out"], dtype=np.float32)

    dt = {"fp32": np.float32, "bf16": ml_dtypes.bfloat16,
          "fp16": np.float16}[dt_flag]

    # x differencing: xd_0 = x_0 ; xd_t = x_t - 0.8 x_{t-1}
    xd = x.copy()
    xd[:, 1:] -= DECAY * x[:, :-1]

    wrecT = np.ascontiguousarray((SCALE * ALPHA * W_rec.T).astype(dt))
    winT = np.ascontiguousarray((SCALE * W_in.T).astype(dt))
    id08 = np.ascontiguousarray((DECAY * np.eye(128)).astype(dt))
    beta_v = np.ascontiguousarray(
        (SCALE * (b_in + h_bias)).astype(dt).reshape(1, H))
    with_beta = bool(np.any(np.asarray(beta_v, dtype=np.float32) != 0))
    woutT = np.ascontiguousarray((ALPHA * W_out.T).astype(dt))
    bout = np.ascontiguousarray(b_out.reshape(O, 1))

    in_maps = []
    for c in range(NCORES):
        xc = xd[c * BL:(c + 1) * BL]                    # [BL, T, I]
        xTc = np.ascontiguousarray(
            xc.transpose(2, 1, 0).reshape(I, T * BL).astype(dt))
        in_maps.append({
            "xT": xTc, "wrecT": wrecT, "winT": winT, "id08": id08,
            "beta": beta_v, "woutT": woutT, "bout": bout,
        })
    return in_maps, with_beta


def _run(inputs, trace=False, dt_flag=None, tmpdir=None):
    from concourse import bass_utils

    if dt_flag is None:
        dt_flag = DT_REC
    in_maps, with_beta = _prep_in_maps(inputs, dt_flag)
    nc = _get_nc(dt_flag, with_beta)
    res = bass_utils.run_bass_kernel_spmd(
        nc, in_maps, core_ids=list(range(NCORES)), trace=trace, tmpdir=tmpdir,
    )
    outs = []
    for c in range(NCORES):
        oT = res.results[c]["outT"]                     # [O, T*BL]
        outs.append(oT.reshape(O, T, BL).transpose(2, 1, 0))
    full = np.concatenate(outs, axis=0).astype(np.float32)
    return full, res


def kernel(**inputs) -> np.ndarray:
    out, _ = _run(inputs, trace=False)
    return out
